# revision 13
# baseline (speedup 1.0000x reference)
"""Trainium2 Bass kernel for CrossAttentionFusion (v6).

Reference computation (per batch b):
    Q = q_w @ f1 + q_b          (O, N)   f1 = features1[b] as (C, N)
    K = k_w @ f2 + k_b          (O, N)
    V = v_w @ f2 + v_b          -> used as (N, O)
    A = softmax(Q^T K / sqrt(O))  over keys          (N, N)
    att = A @ V                  (N, O)
    Z = o_w @ att^T + o_b        (O, N)
    out = GroupNorm(8 groups over O, spatial N)(Z) * gn_w + gn_b

Sharding: pure data-parallel, batch b -> NeuronCore b (B=8, 8 cores).

Key structural points (v6):
 * Score reassociation: softmax is invariant to per-query shifts, so
       S'[k,q] = f2[:,k]^T G[:,q] + r_k,   G = (q_w^T k_w)^T f1,
       r_k = (k_w^T q_b)^T f2[:,k] * scale
   K and Q projections disappear; r_k rides as a 257th output column
   of the V projection and enters exp() via the activation bias port.
 * Uniform 512-wide query chunks (a, b, s, c1, c2). Chunk-a scores
   start right after the first G chunk; each later chunk's scores
   interleave into the previous chunk's attention compute.
 * Startup is DMA-bandwidth-bound (~2.8 MB at ~240 B/ns): inputs are
   cut into just-in-time pieces ordered by first consumption; PE runs
   warm-up matmuls on scratch during the DMA dead zone so the p-state
   ramp completes before real work arrives.
 * Denominator: inline pairwise bf16 tree -> ones-vector matmul ->
   DVE reciprocal -> gpsimd partition_broadcast.
 * GN tail: gn_w/gn_b folded into the bf16 broadcast matmul
   (gselTw: rows 0-7 one-hot*gn_w, row 8 = gn_b; ms2 = [rstd,
   mean*rstd | 0,-1]) so pst directly yields [a_col | t_col];
   rstd = exp(-0.5 ln(var+eps)) on the idle scalar engine (ln/exp
   share the loaded activation table with Square/Copy).
 * O projection + GN matmuls bf16; Z and output bf16 (host -> f32).
"""

import numpy as np

B = 8
C = 256
O = 256
N = 2304
NKT = 18  # key tiles of 128
VW = O + 1  # V projection width: O cols of V + 1 col of r_k*scale
GROUPS = 8
EPS = 1e-5
SCALE = float(O) ** -0.5

_BUILD_CACHE = {}


def _build_nc():
    import concourse.mybir as mybir
    import concourse.tile as tile
    from concourse import bacc
    from concourse.bass import ts

    F32 = mybir.dt.float32
    BF16 = mybir.dt.bfloat16
    I32 = mybir.dt.int32
    AF = mybir.ActivationFunctionType
    ALU = mybir.AluOpType

    nc = bacc.Bacc("TRN2", target_bir_lowering=False)

    f1_d = nc.dram_tensor("features1", [C, N], BF16, kind="ExternalInput")
    f2_d = nc.dram_tensor("features2", [C, N], BF16, kind="ExternalInput")
    # host-precomputed: HT = q_w.T @ k_w (lhsT for G), vwT_aug = [v_w.T | k_w.T q_b * scale]
    ht_d = nc.dram_tensor("HT", [C, O], BF16, kind="ExternalInput")
    vwT_d = nc.dram_tensor("vwT_aug", [C, VW], BF16, kind="ExternalInput")
    owT_d = nc.dram_tensor("owT", [O, O], BF16, kind="ExternalInput")
    vbr_d = nc.dram_tensor("vb_row", [1, VW], F32, kind="ExternalInput")
    ob_d = nc.dram_tensor("o_b", [O], F32, kind="ExternalInput")
    gsel_d = nc.dram_tensor("gsel", [128, 2 * GROUPS], F32, kind="ExternalInput")
    gselw_d = nc.dram_tensor(
        "gselTw", [GROUPS + 1, 2 * 128], BF16, kind="ExternalInput"
    )
    ms2i_d = nc.dram_tensor("ms2_init", [GROUPS + 1, 2], BF16, kind="ExternalInput")
    onesb_d = nc.dram_tensor("ones_bf", [128, 1], BF16, kind="ExternalInput")
    out_d = nc.dram_tensor("out", [O, N], BF16, kind="ExternalOutput")

    with tile.TileContext(nc) as tc:
        with (
            tc.tile_pool(name="consts", bufs=1) as consts,
            tc.tile_pool(name="weights", bufs=1) as wpool,
            tc.tile_pool(name="acts", bufs=1) as apool,
            tc.tile_pool(name="feat", bufs=1) as fpool,
            tc.tile_pool(name="ppool", bufs=2) as ppool,
            tc.tile_pool(name="tpool", bufs=1) as tpool,
            tc.tile_pool(name="sbm", bufs=2) as sbm,
        ):
            # ---- persistent tiles ----
            ht = [wpool.tile([128, O], BF16, name=f"ht{t}") for t in range(2)]
            vwT = [wpool.tile([128, VW], BF16, name=f"vwT{t}") for t in range(2)]
            owT = [wpool.tile([128, O], BF16, name=f"owT{t}") for t in range(2)]
            warm = wpool.tile([128, 512], BF16, name="warm")
            vb_row = consts.tile([1, VW], F32, name="vb_row")
            vb_bc = consts.tile([128, VW], F32, name="vb_bc")
            ones_bf = consts.tile([128, 1], BF16, name="ones_bf")
            gsel = consts.tile([128, 2 * GROUPS], F32, name="gsel")
            gselTw = consts.tile([GROUPS + 1, 2 * 128], BF16, name="gselTw")
            ms2 = consts.tile([GROUPS + 1, 2], BF16, name="ms2")
            ob_c = [consts.tile([128, 1], F32, name=f"ob{t}") for t in range(2)]

            f1sb = [fpool.tile([128, N], BF16, name=f"f1sb{t}") for t in range(2)]
            f2sb = [apool.tile([128, N], BF16, name=f"f2sb{t}") for t in range(2)]
            G = [apool.tile([128, N], BF16, name=f"G{t}") for t in range(2)]
            V = [apool.tile([128, VW], BF16, name=f"V{k}") for k in range(NKT)]
            Z = [apool.tile([128, N], BF16, name=f"Z{t}") for t in range(2)]
            st_sums = [apool.tile([128, 2], F32, name=f"st{t}") for t in range(2)]
            for t in range(2):
                nc.vector.memset(st_sums[t], 0.0)
            nc.vector.memset(warm, 0.0)

            # ---- DMA issue: just-in-time pieces ordered by consumption ----
            # sync ring (C-half 0): ht, f1 head, f2 in 384-col steps, f1 mid,
            # f2 tail halves
            nc.sync.dma_start(out=ht[0], in_=ht_d[ts(0, 128), :])
            nc.sync.dma_start(out=f1sb[0][:, 0:512], in_=f1_d[ts(0, 128), 0:512])
            nc.sync.dma_start(out=f2sb[0][:, 0:384], in_=f2_d[ts(0, 128), 0:384])
            nc.sync.dma_start(out=f2sb[0][:, 384:768], in_=f2_d[ts(0, 128), 384:768])
            nc.sync.dma_start(out=f2sb[0][:, 768:1152], in_=f2_d[ts(0, 128), 768:1152])
            nc.sync.dma_start(out=f1sb[0][:, 512:1024], in_=f1_d[ts(0, 128), 512:1024])
            nc.sync.dma_start(out=f2sb[0][:, 1152:1728], in_=f2_d[ts(0, 128), 1152:1728])
            nc.sync.dma_start(out=f2sb[0][:, 1728:N], in_=f2_d[ts(0, 128), 1728:N])
            # scalar ring (C-half 1): same
            nc.scalar.dma_start(out=ht[1], in_=ht_d[ts(1, 128), :])
            nc.scalar.dma_start(out=f1sb[1][:, 0:512], in_=f1_d[ts(1, 128), 0:512])
            nc.scalar.dma_start(out=f2sb[1][:, 0:384], in_=f2_d[ts(1, 128), 0:384])
            nc.scalar.dma_start(out=f2sb[1][:, 384:768], in_=f2_d[ts(1, 128), 384:768])
            nc.scalar.dma_start(out=f2sb[1][:, 768:1152], in_=f2_d[ts(1, 128), 768:1152])
            nc.scalar.dma_start(
                out=f1sb[1][:, 512:1024], in_=f1_d[ts(1, 128), 512:1024]
            )
            nc.scalar.dma_start(
                out=f2sb[1][:, 1152:1728], in_=f2_d[ts(1, 128), 1152:1728]
            )
            nc.scalar.dma_start(out=f2sb[1][:, 1728:N], in_=f2_d[ts(1, 128), 1728:N])
            # gpsimd: V-proj weights, f1 tails in g-chunk pieces, owT, consts
            for t in range(2):
                nc.gpsimd.dma_start(out=vwT[t], in_=vwT_d[ts(t, 128), :])
            nc.gpsimd.dma_start(out=vb_row, in_=vbr_d[:, :])
            # v_b broadcast built on-device (saves 130 KB of DMA); issued
            # here so it runs as soon as vb_row lands
            nc.gpsimd.partition_broadcast(vb_bc, vb_row)
            for t in range(2):
                nc.gpsimd.dma_start(
                    out=f1sb[t][:, 1024:1536], in_=f1_d[ts(t, 128), 1024:1536]
                )
            for t in range(2):
                nc.gpsimd.dma_start(
                    out=f1sb[t][:, 1536:2048], in_=f1_d[ts(t, 128), 1536:2048]
                )
            for t in range(2):
                nc.gpsimd.dma_start(out=f1sb[t][:, 2048:N], in_=f1_d[ts(t, 128), 2048:N])
            for t in range(2):
                nc.gpsimd.dma_start(out=owT[t], in_=owT_d[ts(t, 128), :])
            nc.gpsimd.dma_start(out=ones_bf, in_=onesb_d[:, :])
            nc.gpsimd.dma_start(out=gsel, in_=gsel_d[:, :])
            nc.gpsimd.dma_start(out=gselTw, in_=gselw_d[:, :])
            nc.gpsimd.dma_start(out=ms2, in_=ms2i_d[:, :])
            for t in range(2):
                nc.gpsimd.dma_start(out=ob_c[t], in_=ob_d[ts(t, 128)].unsqueeze(1))

            # G chunks: (start, width) in query-column space
            GCH = [(0, 512), (512, 512), (1024, 512), (1536, 512), (2048, 256)]

            with tc.tile_pool(name="sps", bufs=3, space="PSUM") as sps:

                def scores_nk(j0, jw, nk):
                    sp = sps.tile([128, 512], F32, tag="sp", name="sp")
                    nc.tensor.matmul(
                        sp[:, :jw],
                        f2sb[0][:, ts(nk, 128)],
                        G[0][:, j0 : j0 + jw],
                        start=True,
                        stop=False,
                    )
                    nc.tensor.matmul(
                        sp[:, :jw],
                        f2sb[1][:, ts(nk, 128)],
                        G[1][:, j0 : j0 + jw],
                        start=False,
                        stop=True,
                    )
                    pt = ppool.tile([128, 512], BF16, tag=f"p{nk}", name=f"pt{nk}")
                    nc.scalar.activation(
                        pt[:, :jw],
                        sp[:, :jw],
                        AF.Exp,
                        bias=V[nk][:, O : O + 1],
                        scale=SCALE,
                    )
                    return pt

                # pairwise tree: first level emitted inline with the score
                # loop (pair_add), the rest by tree_fin.
                def tree_tiles(pref, jw):
                    return [
                        tpool.tile(
                            [128, jw], BF16, tag=f"tr{pref}{i}", name=f"tr{pref}{i}"
                        )
                        for i in range(9)
                    ]

                def pair_add(tr, P, i, jw):
                    nc.vector.tensor_add(
                        tr[i][:, :jw], P[2 * i][:, :jw], P[2 * i + 1][:, :jw]
                    )

                def tree_fin(tr, jw):
                    for i in range(4):
                        nc.vector.tensor_add(
                            tr[2 * i][:, :jw], tr[2 * i][:, :jw], tr[2 * i + 1][:, :jw]
                        )
                    nc.vector.tensor_add(tr[0][:, :jw], tr[0][:, :jw], tr[2][:, :jw])
                    nc.vector.tensor_add(tr[4][:, :jw], tr[4][:, :jw], tr[6][:, :jw])
                    nc.vector.tensor_add(tr[0][:, :jw], tr[0][:, :jw], tr[4][:, :jw])
                    nc.vector.tensor_add(tr[0][:, :jw], tr[0][:, :jw], tr[8][:, :jw])
                    return tr[0]

                # ---- phase A: warm-up + G chunks + V proj + chunk-a scores ----
                Pa = []
                tr_a = tree_tiles("a", 512)
                with (
                    tc.tile_pool(name="vps", bufs=2, space="PSUM") as vps,
                    tc.tile_pool(name="gpsA", bufs=2, space="PSUM") as gpsA,
                    tc.tile_pool(name="wps", bufs=1, space="PSUM") as wps,
                ):
                    # p-state warm-up on scratch while input DMA streams in
                    wp = wps.tile([128, 256], F32, tag="warm", name="wp")
                    for _ in range(11):
                        nc.tensor.matmul(
                            wp, warm[:, 0:128], warm[:, 0:256],
                            start=True, stop=True,
                        )

                    def g_chunk(ci, eng):
                        c0, cw = GCH[ci]
                        csl = slice(c0, c0 + cw)
                        for t in range(2):
                            gp = gpsA.tile([128, 512], F32, tag="gp", name="gp")
                            nc.tensor.matmul(
                                gp[:, :cw], ht[0][:, ts(t, 128)], f1sb[0][:, csl],
                                start=True, stop=False,
                            )
                            nc.tensor.matmul(
                                gp[:, :cw], ht[1][:, ts(t, 128)], f1sb[1][:, csl],
                                start=False, stop=True,
                            )
                            # evac split across DVE/scalar to balance load
                            if eng == "v":
                                nc.vector.tensor_scalar_mul(
                                    G[t][:, csl], gp[:, :cw], 1.0
                                )
                            else:
                                nc.scalar.copy(G[t][:, csl], gp[:, :cw])

                    def v_tile(nk):
                        vp = vps.tile([128, VW], F32, tag="vp", name="vp")
                        nc.tensor.matmul(
                            vp, f2sb[0][:, ts(nk, 128)], vwT[0], start=True, stop=False
                        )
                        nc.tensor.matmul(
                            vp, f2sb[1][:, ts(nk, 128)], vwT[1], start=False, stop=True
                        )
                        nc.vector.tensor_add(V[nk], vp, vb_bc)

                    g_chunk(0, "v")
                    for nk in range(NKT):
                        v_tile(nk)
                        Pa.append(scores_nk(0, 512, nk))
                        if nk % 2 == 1:
                            pair_add(tr_a, Pa, nk // 2, 512)
                        if nk == 7:
                            g_chunk(1, "s")
                        elif nk == 10:
                            g_chunk(2, "v")
                        elif nk == 12:
                            g_chunk(3, "s")
                        elif nk == 14:
                            g_chunk(4, "v")
                tr0_a = tree_fin(tr_a, 512)

                # ---- phase B ----
                with (
                    tc.tile_pool(name="ops", bufs=2, space="PSUM") as ops,
                    tc.tile_pool(name="zps", bufs=1, space="PSUM") as zps,
                    tc.tile_pool(name="dps", bufs=1, space="PSUM") as dps,
                ):

                    def denom(tr0, s0, sw):
                        ssl = slice(s0, s0 + sw)
                        dn = dps.tile([1, 512], F32, tag="d", name="dn")
                        nc.tensor.matmul(
                            dn[:, :sw], ones_bf, tr0[:, ssl], start=True, stop=True
                        )
                        rrow = sbm.tile([1, 512], F32, tag="rrow", name="rrow")
                        nc.vector.reciprocal_approx_fast(rrow[:, :sw], dn[:, :sw])
                        bcs = sbm.tile([128, 512], F32, tag="bcs", name="bcs")
                        nc.gpsimd.partition_broadcast(bcs[:, :sw], rrow[:, :sw])
                        return bcs

                    def attn_o(P, s0, sw, o):
                        ssl = slice(s0, s0 + sw)
                        op = ops.tile([128, 512], F32, tag="op", name="op")
                        for nk in range(NKT):
                            nc.tensor.matmul(
                                op[:, :sw],
                                V[nk][:, ts(o, 128)],
                                P[nk][:, ssl],
                                start=(nk == 0),
                                stop=(nk == NKT - 1),
                            )
                        return op

                    def comp_fin(j0, oacc, bcs, s0, sw):
                        ATs = []
                        for o in range(2):
                            at = sbm.tile([128, 512], BF16, tag=f"at{o}", name=f"at{o}")
                            nc.vector.tensor_mul(
                                at[:, :sw], oacc[o][:, :sw], bcs[:, :sw]
                            )
                            ATs.append(at)
                        # output projection sub-chunk: Z[p, sw]
                        zsl = slice(j0 + s0, j0 + s0 + sw)
                        for p in range(2):
                            zp = zps.tile([128, 512], F32, tag="zp", name="zp")
                            nc.tensor.matmul(
                                zp[:, :sw], owT[0][:, ts(p, 128)], ATs[0][:, :sw],
                                start=True, stop=False,
                            )
                            nc.tensor.matmul(
                                zp[:, :sw], owT[1][:, ts(p, 128)], ATs[1][:, :sw],
                                start=False, stop=True,
                            )
                            part = sbm.tile(
                                [128, 2], F32, tag=f"part{p}", name=f"part{p}"
                            )
                            nc.vector.tensor_scalar(
                                Z[p][:, zsl],
                                zp[:, :sw],
                                ob_c[p],
                                0.0,
                                op0=ALU.add,
                                op1=ALU.add,
                                accum_out=part[:, 0:1],
                            )
                            sqs = sbm.tile([128, 512], BF16, tag="sqs", name="sqs")
                            nc.scalar.activation(
                                sqs[:, :sw],
                                Z[p][:, zsl],
                                AF.Square,
                                accum_out=part[:, 1:2],
                            )
                            nc.vector.tensor_add(st_sums[p], st_sums[p], part)

                    # chunk b scores interleave into chunk-a attention, etc.
                    Pb, Ps, Pc1, Pc2 = [], [], [], []
                    tr_b = tree_tiles("b", 512)
                    tr_s = tree_tiles("s", 256)
                    tr_c1 = tree_tiles("c1", 512)
                    tr_c2 = tree_tiles("c2", 512)

                    def sweep(P, tr, j0, jw, a, b):
                        for nk in range(a, b):
                            P.append(scores_nk(j0, jw, nk))
                            if nk % 2 == 1:
                                pair_add(tr, P, nk // 2, jw)

                    # step 1: attn-a || scores-b
                    op_a0 = attn_o(Pa, 0, 512, 0)
                    sweep(Pb, tr_b, 512, 512, 0, 6)
                    bcs_a = denom(tr0_a, 0, 512)
                    op_a1 = attn_o(Pa, 0, 512, 1)
                    sweep(Pb, tr_b, 512, 512, 6, 12)
                    comp_fin(0, [op_a0, op_a1], bcs_a, 0, 512)
                    sweep(Pb, tr_b, 512, 512, 12, 18)
                    tr0_b = tree_fin(tr_b, 512)

                    # step 2: attn-b || scores-s
                    op_b0 = attn_o(Pb, 0, 512, 0)
                    sweep(Ps, tr_s, 2048, 256, 0, 6)
                    bcs_b = denom(tr0_b, 0, 512)
                    op_b1 = attn_o(Pb, 0, 512, 1)
                    sweep(Ps, tr_s, 2048, 256, 6, 12)
                    comp_fin(512, [op_b0, op_b1], bcs_b, 0, 512)
                    sweep(Ps, tr_s, 2048, 256, 12, 18)
                    tr0_s = tree_fin(tr_s, 256)

                    # step 3: attn-s || scores-c1
                    sweep(Pc1, tr_c1, 1024, 512, 0, 5)
                    op_s0 = attn_o(Ps, 0, 256, 0)
                    sweep(Pc1, tr_c1, 1024, 512, 5, 10)
                    bcs_s = denom(tr0_s, 0, 256)
                    op_s1 = attn_o(Ps, 0, 256, 1)
                    sweep(Pc1, tr_c1, 1024, 512, 10, 14)
                    comp_fin(2048, [op_s0, op_s1], bcs_s, 0, 256)
                    sweep(Pc1, tr_c1, 1024, 512, 14, 18)
                    tr0_c1 = tree_fin(tr_c1, 512)

                    # step 4: attn-c1 || scores-c2
                    op_c10 = attn_o(Pc1, 0, 512, 0)
                    sweep(Pc2, tr_c2, 1536, 512, 0, 6)
                    bcs_c1 = denom(tr0_c1, 0, 512)
                    op_c11 = attn_o(Pc1, 0, 512, 1)
                    sweep(Pc2, tr_c2, 1536, 512, 6, 12)
                    comp_fin(1024, [op_c10, op_c11], bcs_c1, 0, 512)
                    sweep(Pc2, tr_c2, 1536, 512, 12, 18)
                    tr0_c2 = tree_fin(tr_c2, 512)

                    # step 5: attn-c2 in 384 + 128 col blocks (short GN tail)
                    op_w0 = attn_o(Pc2, 0, 384, 0)
                    bcs_w = denom(tr0_c2, 0, 384)
                    op_w1 = attn_o(Pc2, 0, 384, 1)
                    comp_fin(1536, [op_w0, op_w1], bcs_w, 0, 384)
                    op_x0 = attn_o(Pc2, 384, 128, 0)
                    bcs_x = denom(tr0_c2, 384, 128)
                    op_x1 = attn_o(Pc2, 384, 128, 1)
                    comp_fin(1536, [op_x0, op_x1], bcs_x, 384, 128)

            # ---- phase C: GroupNorm finalization ----
            with (
                tc.tile_pool(name="gns", bufs=2) as gns,
                tc.tile_pool(name="gout", bufs=1) as gout,
                tc.tile_pool(name="gps", bufs=2, space="PSUM") as gps,
            ):
                # gsel is pre-scaled by 1/(32*N) on host: gst = [mean, E[x^2]]
                gst = gps.tile([GROUPS, 2], F32, tag="gst", name="gst")
                nc.tensor.matmul(
                    gst, gsel[:, 0:GROUPS], st_sums[0], start=True, stop=False
                )
                nc.tensor.matmul(
                    gst,
                    gsel[:, GROUPS : 2 * GROUPS],
                    st_sums[1],
                    start=False,
                    stop=True,
                )
                msf = gns.tile([GROUPS, 2], F32, tag="msf", name="msf")
                nc.vector.tensor_scalar_mul(msf, gst, 1.0)  # PSUM -> SBUF
                m2 = gns.tile([GROUPS, 1], F32, tag="m2", name="m2")
                nc.vector.tensor_mul(m2, msf[:, 0:1], msf[:, 0:1])  # mean^2
                ve = gns.tile([GROUPS, 1], F32, tag="ve", name="ve")
                nc.vector.scalar_tensor_tensor(
                    ve, msf[:, 1:2], EPS, m2, op0=ALU.add, op1=ALU.subtract
                )  # var+eps
                # rstd = exp(-0.5*ln(var+eps)) on the (idle) scalar engine;
                # ln/exp live in the same act table as Square/Copy.
                lv = gns.tile([GROUPS, 1], F32, tag="lv", name="lv")
                nc.scalar.activation(lv, ve, AF.Ln)
                rs = gns.tile([GROUPS, 1], F32, tag="rs", name="rs")
                nc.scalar.activation(rs, lv, AF.Exp, scale=-0.5)
                # ms2 rows 0-7 = [rstd, mean*rstd]; row 8 = [0, -1] (from DMA)
                nc.vector.tensor_scalar_mul(ms2[0:GROUPS, 0:1], rs, 1.0)
                nc.vector.tensor_mul(ms2[0:GROUPS, 1:2], rs, msf[:, 0:1])
                ats = []
                for p in range(2):
                    pst = gps.tile([128, 2], F32, tag="pst", name="pst")
                    nc.tensor.matmul(
                        pst, gselTw[:, ts(p, 128)], ms2, start=True, stop=True
                    )
                    at_sb = gns.tile([128, 2], F32, tag=f"at_sb{p}", name="at_sb")
                    nc.vector.tensor_scalar_mul(at_sb, pst, 1.0)
                    ats.append(at_sb)
                # out = Z*a - t on DVE; DMA fired right after each piece on
                # sync (p0) / scalar (p1) rings
                for hi in range(2):
                    h0 = 1152 * hi
                    hsl = slice(h0, h0 + 1152)
                    for p in range(2):
                        outp = gout.tile(
                            [128, 1152], BF16, tag=f"outp{p}{hi}", name="outp"
                        )
                        nc.vector.tensor_scalar(
                            outp,
                            Z[p][:, hsl],
                            ats[p][:, 0:1],
                            ats[p][:, 1:2],
                            op0=ALU.mult,
                            op1=ALU.subtract,
                        )
                        ring = nc.sync if p == 0 else nc.scalar
                        ring.dma_start(out=out_d[ts(p, 128), hsl], in_=outp)

    nc.finalize()
    return nc


def _get_nc():
    if "nc" not in _BUILD_CACHE:
        _BUILD_CACHE["nc"] = _build_nc()
    return _BUILD_CACHE["nc"]


def _make_in_maps(inputs):
    import ml_dtypes

    f1 = np.ascontiguousarray(
        np.asarray(inputs["features1"], dtype=np.float32)
        .reshape(B, C, N)
        .astype(ml_dtypes.bfloat16)
    )
    f2 = np.ascontiguousarray(
        np.asarray(inputs["features2"], dtype=np.float32)
        .reshape(B, C, N)
        .astype(ml_dtypes.bfloat16)
    )

    def g(k):
        return np.asarray(inputs[k], dtype=np.float32)

    inv_cnt = 1.0 / (32.0 * N)
    gsel = np.zeros((128, 2 * GROUPS), np.float32)
    # gselTw: rows 0-7 one-hot * gn_w, row 8 = gn_b (col layout [t*128+c])
    gselTw = np.zeros((GROUPS + 1, 2 * 128), np.float32)
    gnw, gnb = g("gn_w"), g("gn_b")
    for t in range(2):
        for gl in range(4):
            grp = 4 * t + gl
            gsel[gl * 32 : (gl + 1) * 32, GROUPS * t + grp] = inv_cnt
            cs = 128 * t + gl * 32
            gselTw[grp, cs : cs + 32] = gnw[grp * 32 : (grp + 1) * 32]
        gselTw[GROUPS, 128 * t : 128 * (t + 1)] = gnb[128 * t : 128 * (t + 1)]
    ms2_init = np.zeros((GROUPS + 1, 2), np.float32)
    ms2_init[GROUPS] = [0.0, -1.0]

    qw, kw, vw = g("q_w"), g("k_w"), g("v_w")
    HT = np.ascontiguousarray((qw.T @ kw).astype(ml_dtypes.bfloat16))
    rcol = (kw.T @ g("q_b")) * SCALE  # [C]
    vwT_aug = np.concatenate([vw.T, rcol[:, None]], axis=1)
    vb_row = np.zeros((1, VW), np.float32)
    vb_row[0, :O] = g("v_b")
    shared = {
        "HT": HT,
        "vwT_aug": np.ascontiguousarray(vwT_aug.astype(ml_dtypes.bfloat16)),
        "owT": np.ascontiguousarray(g("o_w").T.astype(ml_dtypes.bfloat16)),
        "vb_row": vb_row,
        "o_b": g("o_b"),
        "gsel": gsel,
        "gselTw": np.ascontiguousarray(gselTw.astype(ml_dtypes.bfloat16)),
        "ms2_init": np.ascontiguousarray(ms2_init.astype(ml_dtypes.bfloat16)),
        "ones_bf": np.ones((128, 1), ml_dtypes.bfloat16),
    }
    return [{"features1": f1[i], "features2": f2[i], **shared} for i in range(B)]


def run(inputs, trace=False):
    from concourse.bass_utils import run_bass_kernel_spmd

    nc = _get_nc()
    in_maps = _make_in_maps(inputs)
    res = run_bass_kernel_spmd(nc, in_maps, core_ids=list(range(B)), trace=trace)
    out = np.stack(
        [np.asarray(res.results[i]["out"]).astype(np.float32) for i in range(B)]
    )
    return out.reshape(B, O, 48, 48), res


def kernel(**inputs):
    out, _ = run(inputs, trace=False)
    return out


# revision 19
# speedup vs baseline: 1.0356x; 1.0356x over previous
"""Trainium2 Bass kernel for CrossAttentionFusion (v6).

Reference computation (per batch b):
    Q = q_w @ f1 + q_b          (O, N)   f1 = features1[b] as (C, N)
    K = k_w @ f2 + k_b          (O, N)
    V = v_w @ f2 + v_b          -> used as (N, O)
    A = softmax(Q^T K / sqrt(O))  over keys          (N, N)
    att = A @ V                  (N, O)
    Z = o_w @ att^T + o_b        (O, N)
    out = GroupNorm(8 groups over O, spatial N)(Z) * gn_w + gn_b

Sharding: pure data-parallel, batch b -> NeuronCore b (B=8, 8 cores).

Key structural points (v6):
 * Score reassociation: softmax is invariant to per-query shifts, so
       S'[k,q] = f2[:,k]^T G[:,q] + r_k,   G = (q_w^T k_w)^T f1,
       r_k = (k_w^T q_b)^T f2[:,k] * scale
   K and Q projections disappear; r_k rides as a 257th output column
   of the V projection and enters exp() via the activation bias port.
 * Uniform 512-wide query chunks (a, b, s, c1, c2). Chunk-a scores
   start right after the first G chunk; each later chunk's scores
   interleave into the previous chunk's attention compute.
 * Startup is DMA-bandwidth-bound (~2.8 MB at ~240 B/ns): inputs are
   cut into just-in-time pieces ordered by first consumption; PE runs
   warm-up matmuls on scratch during the DMA dead zone so the p-state
   ramp completes before real work arrives.
 * Denominator: inline pairwise bf16 tree -> ones-vector matmul ->
   DVE reciprocal -> gpsimd partition_broadcast.
 * GN tail: gn_w/gn_b folded into the bf16 broadcast matmul
   (gselTw: rows 0-7 one-hot*gn_w, row 8 = gn_b; ms2 = [rstd,
   mean*rstd | 0,-1]) so pst directly yields [a_col | t_col];
   rstd = exp(-0.5 ln(var+eps)) on the idle scalar engine (ln/exp
   share the loaded activation table with Square/Copy).
 * O projection + GN matmuls bf16; Z and output bf16 (host -> f32).
"""

import numpy as np

B = 8
C = 256
O = 256
N = 2304
NKT = 18  # key tiles of 128
VW = O + 1  # V projection width: O cols of V + 1 col of r_k*scale
GROUPS = 8
EPS = 1e-5
SCALE = float(O) ** -0.5

_BUILD_CACHE = {}


def _build_nc():
    import concourse.mybir as mybir
    import concourse.tile as tile
    from concourse import bacc
    from concourse.bass import ts

    F32 = mybir.dt.float32
    BF16 = mybir.dt.bfloat16
    I32 = mybir.dt.int32
    AF = mybir.ActivationFunctionType
    ALU = mybir.AluOpType

    nc = bacc.Bacc("TRN2", target_bir_lowering=False)

    f1_d = nc.dram_tensor("features1", [C, N], BF16, kind="ExternalInput")
    f2_d = nc.dram_tensor("features2", [C, N], BF16, kind="ExternalInput")
    # host-precomputed: HT = q_w.T @ k_w (lhsT for G), vwT_aug = [v_w.T | k_w.T q_b * scale]
    ht_d = nc.dram_tensor("HT", [C, O], BF16, kind="ExternalInput")
    vwT_d = nc.dram_tensor("vwT_aug", [C, VW], BF16, kind="ExternalInput")
    owT_d = nc.dram_tensor("owT", [O, O], BF16, kind="ExternalInput")
    vbb_d = nc.dram_tensor("vb_bcast", [128, VW], F32, kind="ExternalInput")
    magic_d = nc.dram_tensor("magic", [GROUPS, 1], I32, kind="ExternalInput")
    ob_d = nc.dram_tensor("o_b", [O], F32, kind="ExternalInput")
    gsel_d = nc.dram_tensor("gsel", [128, 2 * GROUPS], F32, kind="ExternalInput")
    gselw_d = nc.dram_tensor(
        "gselTw", [GROUPS + 1, 2 * 128], BF16, kind="ExternalInput"
    )
    ms2i_d = nc.dram_tensor("ms2_init", [GROUPS + 1, 2], BF16, kind="ExternalInput")
    onesb_d = nc.dram_tensor("ones_bf", [128, 1], BF16, kind="ExternalInput")
    out_d = nc.dram_tensor("out", [O, N], BF16, kind="ExternalOutput")

    with tile.TileContext(nc) as tc:
        with (
            tc.tile_pool(name="consts", bufs=1) as consts,
            tc.tile_pool(name="weights", bufs=1) as wpool,
            tc.tile_pool(name="acts", bufs=1) as apool,
            tc.tile_pool(name="feat", bufs=1) as fpool,
            tc.tile_pool(name="ppool", bufs=2) as ppool,
            tc.tile_pool(name="tpool", bufs=1) as tpool,
            tc.tile_pool(name="sbm", bufs=2) as sbm,
        ):
            # ---- persistent tiles ----
            ht = [wpool.tile([128, O], BF16, name=f"ht{t}") for t in range(2)]
            vwT = [wpool.tile([128, VW], BF16, name=f"vwT{t}") for t in range(2)]
            owT = [wpool.tile([128, O], BF16, name=f"owT{t}") for t in range(2)]
            warm = wpool.tile([128, 512], BF16, name="warm")
            vb_bc = consts.tile([128, VW], F32, name="vb_bc")
            magic_c = consts.tile([GROUPS, 1], I32, name="magic_c")
            ones_bf = consts.tile([128, 1], BF16, name="ones_bf")
            gsel = consts.tile([128, 2 * GROUPS], F32, name="gsel")
            gselTw = consts.tile([GROUPS + 1, 2 * 128], BF16, name="gselTw")
            ms2 = consts.tile([GROUPS + 1, 2], BF16, name="ms2")
            ob_c = [consts.tile([128, 1], F32, name=f"ob{t}") for t in range(2)]

            f1sb = [fpool.tile([128, N], BF16, name=f"f1sb{t}") for t in range(2)]
            f2sb = [apool.tile([128, N], BF16, name=f"f2sb{t}") for t in range(2)]
            G = [apool.tile([128, N], BF16, name=f"G{t}") for t in range(2)]
            V = [apool.tile([128, VW], BF16, name=f"V{k}") for k in range(NKT)]
            Z = [apool.tile([128, N], BF16, name=f"Z{t}") for t in range(2)]
            st_sums = [apool.tile([128, 2], F32, name=f"st{t}") for t in range(2)]
            for t in range(2):
                nc.vector.memset(st_sums[t], 0.0)
            nc.vector.memset(warm, 0.0)

            # ---- DMA issue: critical-path operands lead each ring ----
            # sync ring: ht0, f1[0] head, f2[0] pieces, f1[0] mid, f2[0] tail
            nc.sync.dma_start(out=ht[0], in_=ht_d[ts(0, 128), :])
            nc.sync.dma_start(out=f1sb[0][:, 0:512], in_=f1_d[ts(0, 128), 0:512])
            nc.sync.dma_start(out=f2sb[0][:, 0:384], in_=f2_d[ts(0, 128), 0:384])
            nc.sync.dma_start(out=f2sb[0][:, 384:1152], in_=f2_d[ts(0, 128), 384:1152])
            nc.sync.dma_start(out=f1sb[0][:, 512:1024], in_=f1_d[ts(0, 128), 512:1024])
            nc.sync.dma_start(out=f2sb[0][:, 1152:N], in_=f2_d[ts(0, 128), 1152:N])
            # scalar ring: same for the second C-half
            nc.scalar.dma_start(out=ht[1], in_=ht_d[ts(1, 128), :])
            nc.scalar.dma_start(out=f1sb[1][:, 0:512], in_=f1_d[ts(1, 128), 0:512])
            nc.scalar.dma_start(out=f2sb[1][:, 0:384], in_=f2_d[ts(1, 128), 0:384])
            nc.scalar.dma_start(out=f2sb[1][:, 384:1152], in_=f2_d[ts(1, 128), 384:1152])
            nc.scalar.dma_start(
                out=f1sb[1][:, 512:1024], in_=f1_d[ts(1, 128), 512:1024]
            )
            nc.scalar.dma_start(out=f2sb[1][:, 1152:N], in_=f2_d[ts(1, 128), 1152:N])
            # gpsimd: V-proj weights, f1 tails, owT, late consts
            for t in range(2):
                nc.gpsimd.dma_start(out=vwT[t], in_=vwT_d[ts(t, 128), :])
            nc.gpsimd.dma_start(out=vb_bc, in_=vbb_d[:, :])
            nc.gpsimd.dma_start(out=f1sb[0][:, 1024:N], in_=f1_d[ts(0, 128), 1024:N])
            nc.gpsimd.dma_start(out=f1sb[1][:, 1024:N], in_=f1_d[ts(1, 128), 1024:N])
            for t in range(2):
                nc.gpsimd.dma_start(out=owT[t], in_=owT_d[ts(t, 128), :])
            nc.gpsimd.dma_start(out=ones_bf, in_=onesb_d[:, :])
            nc.gpsimd.dma_start(out=gsel, in_=gsel_d[:, :])
            nc.gpsimd.dma_start(out=gselTw, in_=gselw_d[:, :])
            nc.gpsimd.dma_start(out=ms2, in_=ms2i_d[:, :])
            nc.gpsimd.dma_start(out=magic_c, in_=magic_d[:, :])
            for t in range(2):
                nc.gpsimd.dma_start(out=ob_c[t], in_=ob_d[ts(t, 128)].unsqueeze(1))

            # G chunks: (start, width) in query-column space
            GCH = [(0, 512), (512, 512), (1024, 512), (1536, 512), (2048, 256)]

            with tc.tile_pool(name="sps", bufs=3, space="PSUM") as sps:

                def scores_nk(j0, jw, nk):
                    sp = sps.tile([128, 512], F32, tag="sp", name="sp")
                    nc.tensor.matmul(
                        sp[:, :jw],
                        f2sb[0][:, ts(nk, 128)],
                        G[0][:, j0 : j0 + jw],
                        start=True,
                        stop=False,
                    )
                    nc.tensor.matmul(
                        sp[:, :jw],
                        f2sb[1][:, ts(nk, 128)],
                        G[1][:, j0 : j0 + jw],
                        start=False,
                        stop=True,
                    )
                    pt = ppool.tile([128, 512], BF16, tag=f"p{nk}", name=f"pt{nk}")
                    nc.scalar.activation(
                        pt[:, :jw],
                        sp[:, :jw],
                        AF.Exp,
                        bias=V[nk][:, O : O + 1],
                        scale=SCALE,
                    )
                    return pt

                # pairwise tree: first level emitted inline with the score
                # loop (pair_add), the rest by tree_fin.
                def tree_tiles(pref, jw):
                    return [
                        tpool.tile(
                            [128, jw], BF16, tag=f"tr{pref}{i}", name=f"tr{pref}{i}"
                        )
                        for i in range(9)
                    ]

                def pair_add(tr, P, i, jw):
                    nc.vector.tensor_add(
                        tr[i][:, :jw], P[2 * i][:, :jw], P[2 * i + 1][:, :jw]
                    )

                def tree_fin(tr, jw):
                    for i in range(4):
                        nc.vector.tensor_add(
                            tr[2 * i][:, :jw], tr[2 * i][:, :jw], tr[2 * i + 1][:, :jw]
                        )
                    nc.vector.tensor_add(tr[0][:, :jw], tr[0][:, :jw], tr[2][:, :jw])
                    nc.vector.tensor_add(tr[4][:, :jw], tr[4][:, :jw], tr[6][:, :jw])
                    nc.vector.tensor_add(tr[0][:, :jw], tr[0][:, :jw], tr[4][:, :jw])
                    nc.vector.tensor_add(tr[0][:, :jw], tr[0][:, :jw], tr[8][:, :jw])
                    return tr[0]

                # ---- phase A: warm-up + G chunks + V proj + chunk-a scores ----
                Pa = []
                tr_a = tree_tiles("a", 512)
                with (
                    tc.tile_pool(name="vps", bufs=2, space="PSUM") as vps,
                    tc.tile_pool(name="gpsA", bufs=2, space="PSUM") as gpsA,
                    tc.tile_pool(name="wps", bufs=1, space="PSUM") as wps,
                ):
                    # p-state warm-up on scratch while input DMA streams in
                    wp = wps.tile([128, 256], F32, tag="warm", name="wp")
                    for _ in range(11):
                        nc.tensor.matmul(
                            wp, warm[:, 0:128], warm[:, 0:256],
                            start=True, stop=True,
                        )

                    def g_chunk(ci, eng):
                        c0, cw = GCH[ci]
                        csl = slice(c0, c0 + cw)
                        for t in range(2):
                            gp = gpsA.tile([128, 512], F32, tag="gp", name="gp")
                            nc.tensor.matmul(
                                gp[:, :cw], ht[0][:, ts(t, 128)], f1sb[0][:, csl],
                                start=True, stop=False,
                            )
                            nc.tensor.matmul(
                                gp[:, :cw], ht[1][:, ts(t, 128)], f1sb[1][:, csl],
                                start=False, stop=True,
                            )
                            # evac split across DVE/scalar to balance load
                            if eng == "v":
                                nc.vector.tensor_scalar_mul(
                                    G[t][:, csl], gp[:, :cw], 1.0
                                )
                            else:
                                nc.scalar.copy(G[t][:, csl], gp[:, :cw])

                    def v_tile(nk):
                        vp = vps.tile([128, VW], F32, tag="vp", name="vp")
                        nc.tensor.matmul(
                            vp, f2sb[0][:, ts(nk, 128)], vwT[0], start=True, stop=False
                        )
                        nc.tensor.matmul(
                            vp, f2sb[1][:, ts(nk, 128)], vwT[1], start=False, stop=True
                        )
                        nc.vector.tensor_add(V[nk], vp, vb_bc)

                    g_chunk(0, "v")
                    for nk in range(NKT):
                        v_tile(nk)
                        Pa.append(scores_nk(0, 512, nk))
                        if nk % 2 == 1:
                            pair_add(tr_a, Pa, nk // 2, 512)
                        if nk == 6:
                            g_chunk(1, "s")
                        elif nk == 10:
                            g_chunk(2, "s")
                        elif nk == 12:
                            g_chunk(3, "s")
                        elif nk == 14:
                            g_chunk(4, "s")
                tr0_a = tree_fin(tr_a, 512)

                # ---- phase B ----
                with (
                    tc.tile_pool(name="ops", bufs=2, space="PSUM") as ops,
                    tc.tile_pool(name="zps", bufs=1, space="PSUM") as zps,
                    tc.tile_pool(name="dps", bufs=1, space="PSUM") as dps,
                ):

                    def denom(tr0, s0, sw):
                        ssl = slice(s0, s0 + sw)
                        dn = dps.tile([1, 512], F32, tag="d", name="dn")
                        nc.tensor.matmul(
                            dn[:, :sw], ones_bf, tr0[:, ssl], start=True, stop=True
                        )
                        rrow = sbm.tile([1, 512], F32, tag="rrow", name="rrow")
                        nc.vector.reciprocal_approx_fast(rrow[:, :sw], dn[:, :sw])
                        bcs = sbm.tile([128, 512], F32, tag="bcs", name="bcs")
                        nc.gpsimd.partition_broadcast(bcs[:, :sw], rrow[:, :sw])
                        return bcs

                    def attn_o(P, s0, sw, o):
                        ssl = slice(s0, s0 + sw)
                        op = ops.tile([128, 512], F32, tag="op", name="op")
                        for nk in range(NKT):
                            nc.tensor.matmul(
                                op[:, :sw],
                                V[nk][:, ts(o, 128)],
                                P[nk][:, ssl],
                                start=(nk == 0),
                                stop=(nk == NKT - 1),
                            )
                        return op

                    def comp_fin(j0, oacc, bcs, s0, sw):
                        ATs = []
                        for o in range(2):
                            at = sbm.tile([128, 512], BF16, tag=f"at{o}", name=f"at{o}")
                            nc.vector.tensor_mul(
                                at[:, :sw], oacc[o][:, :sw], bcs[:, :sw]
                            )
                            ATs.append(at)
                        # output projection sub-chunk: Z[p, sw]
                        zsl = slice(j0 + s0, j0 + s0 + sw)
                        for p in range(2):
                            zp = zps.tile([128, 512], F32, tag="zp", name="zp")
                            nc.tensor.matmul(
                                zp[:, :sw], owT[0][:, ts(p, 128)], ATs[0][:, :sw],
                                start=True, stop=False,
                            )
                            nc.tensor.matmul(
                                zp[:, :sw], owT[1][:, ts(p, 128)], ATs[1][:, :sw],
                                start=False, stop=True,
                            )
                            part = sbm.tile(
                                [128, 2], F32, tag=f"part{p}", name=f"part{p}"
                            )
                            nc.vector.tensor_scalar(
                                Z[p][:, zsl],
                                zp[:, :sw],
                                ob_c[p],
                                0.0,
                                op0=ALU.add,
                                op1=ALU.add,
                                accum_out=part[:, 0:1],
                            )
                            sqs = sbm.tile([128, 512], BF16, tag="sqs", name="sqs")
                            nc.scalar.activation(
                                sqs[:, :sw],
                                Z[p][:, zsl],
                                AF.Square,
                                accum_out=part[:, 1:2],
                            )
                            nc.vector.tensor_add(st_sums[p], st_sums[p], part)

                    # chunk b scores interleave into chunk-a attention, etc.
                    Pb, Ps, Pc1, Pc2 = [], [], [], []
                    tr_b = tree_tiles("b", 512)
                    tr_s = tree_tiles("s", 256)
                    tr_c1 = tree_tiles("c1", 512)
                    tr_c2 = tree_tiles("c2", 512)

                    def sweep(P, tr, j0, jw, a, b):
                        for nk in range(a, b):
                            P.append(scores_nk(j0, jw, nk))
                            if nk % 2 == 1:
                                pair_add(tr, P, nk // 2, jw)

                    # step 1: attn-a || scores-b
                    op_a0 = attn_o(Pa, 0, 512, 0)
                    sweep(Pb, tr_b, 512, 512, 0, 6)
                    bcs_a = denom(tr0_a, 0, 512)
                    op_a1 = attn_o(Pa, 0, 512, 1)
                    sweep(Pb, tr_b, 512, 512, 6, 12)
                    comp_fin(0, [op_a0, op_a1], bcs_a, 0, 512)
                    sweep(Pb, tr_b, 512, 512, 12, 18)
                    tr0_b = tree_fin(tr_b, 512)

                    # step 2: attn-b || scores-s
                    op_b0 = attn_o(Pb, 0, 512, 0)
                    sweep(Ps, tr_s, 2048, 256, 0, 6)
                    bcs_b = denom(tr0_b, 0, 512)
                    op_b1 = attn_o(Pb, 0, 512, 1)
                    sweep(Ps, tr_s, 2048, 256, 6, 12)
                    comp_fin(512, [op_b0, op_b1], bcs_b, 0, 512)
                    sweep(Ps, tr_s, 2048, 256, 12, 18)
                    tr0_s = tree_fin(tr_s, 256)

                    # step 3: attn-s || scores-c1
                    sweep(Pc1, tr_c1, 1024, 512, 0, 5)
                    op_s0 = attn_o(Ps, 0, 256, 0)
                    sweep(Pc1, tr_c1, 1024, 512, 5, 10)
                    bcs_s = denom(tr0_s, 0, 256)
                    op_s1 = attn_o(Ps, 0, 256, 1)
                    sweep(Pc1, tr_c1, 1024, 512, 10, 14)
                    comp_fin(2048, [op_s0, op_s1], bcs_s, 0, 256)
                    sweep(Pc1, tr_c1, 1024, 512, 14, 18)
                    tr0_c1 = tree_fin(tr_c1, 512)

                    # step 4: attn-c1 || scores-c2
                    op_c10 = attn_o(Pc1, 0, 512, 0)
                    sweep(Pc2, tr_c2, 1536, 512, 0, 6)
                    bcs_c1 = denom(tr0_c1, 0, 512)
                    op_c11 = attn_o(Pc1, 0, 512, 1)
                    sweep(Pc2, tr_c2, 1536, 512, 6, 12)
                    comp_fin(1024, [op_c10, op_c11], bcs_c1, 0, 512)
                    sweep(Pc2, tr_c2, 1536, 512, 12, 18)
                    tr0_c2 = tree_fin(tr_c2, 512)

                    # step 5: attn-c2 in 384 + 128 col blocks (short GN tail)
                    op_w0 = attn_o(Pc2, 0, 384, 0)
                    bcs_w = denom(tr0_c2, 0, 384)
                    op_w1 = attn_o(Pc2, 0, 384, 1)
                    comp_fin(1536, [op_w0, op_w1], bcs_w, 0, 384)
                    op_x0 = attn_o(Pc2, 384, 128, 0)
                    bcs_x = denom(tr0_c2, 384, 128)
                    op_x1 = attn_o(Pc2, 384, 128, 1)
                    comp_fin(1536, [op_x0, op_x1], bcs_x, 384, 128)

            # ---- phase C: GroupNorm finalization ----
            with (
                tc.tile_pool(name="gns", bufs=2) as gns,
                tc.tile_pool(name="gout", bufs=1) as gout,
                tc.tile_pool(name="gps", bufs=2, space="PSUM") as gps,
            ):
                # gsel is pre-scaled by 1/(32*N) on host: gst = [mean, E[x^2]]
                gst = gps.tile([GROUPS, 2], F32, tag="gst", name="gst")
                nc.tensor.matmul(
                    gst, gsel[:, 0:GROUPS], st_sums[0], start=True, stop=False
                )
                nc.tensor.matmul(
                    gst,
                    gsel[:, GROUPS : 2 * GROUPS],
                    st_sums[1],
                    start=False,
                    stop=True,
                )
                msf = gns.tile([GROUPS, 2], F32, tag="msf", name="msf")
                nc.vector.tensor_scalar_mul(msf, gst, 1.0)  # PSUM -> SBUF
                m2 = gns.tile([GROUPS, 1], F32, tag="m2", name="m2")
                nc.vector.tensor_mul(m2, msf[:, 0:1], msf[:, 0:1])  # mean^2
                ve = gns.tile([GROUPS, 1], F32, tag="ve", name="ve")
                nc.vector.scalar_tensor_tensor(
                    ve, msf[:, 1:2], EPS, m2, op0=ALU.add, op1=ALU.subtract
                )  # var+eps
                # rstd = rsqrt(var+eps) on DVE only: bit-trick seed + 1 Newton
                # (Ln on scalar would force an ACT_TABLE_LOAD in the tail)
                sh = gns.tile([GROUPS, 1], I32, tag="sh", name="sh")
                nc.vector.tensor_scalar(
                    sh, ve.bitcast(I32), 1, 0, op0=ALU.arith_shift_right, op1=ALU.bypass
                )
                y0i = gns.tile([GROUPS, 1], I32, tag="y0i", name="y0i")
                nc.vector.tensor_sub(y0i, magic_c, sh)
                y = y0i.bitcast(F32)
                yy = gns.tile([GROUPS, 1], F32, tag="yy", name="yy")
                nc.vector.tensor_mul(yy, y, y)
                nc.vector.tensor_mul(yy, yy, ve)
                nc.vector.tensor_scalar(yy, yy, -0.5, 1.5, op0=ALU.mult, op1=ALU.add)
                rs = gns.tile([GROUPS, 1], F32, tag="rs", name="rs")
                nc.vector.tensor_mul(rs, y, yy)
                # ms2 rows 0-7 = [rstd, mean*rstd]; row 8 = [0, -1] (from DMA)
                nc.vector.tensor_scalar_mul(ms2[0:GROUPS, 0:1], rs, 1.0)
                nc.vector.tensor_mul(ms2[0:GROUPS, 1:2], rs, msf[:, 0:1])
                ats = []
                for p in range(2):
                    pst = gps.tile([128, 2], F32, tag="pst", name="pst")
                    nc.tensor.matmul(
                        pst, gselTw[:, ts(p, 128)], ms2, start=True, stop=True
                    )
                    at_sb = gns.tile([128, 2], F32, tag=f"at_sb{p}", name="at_sb")
                    nc.vector.tensor_scalar_mul(at_sb, pst, 1.0)
                    ats.append(at_sb)
                # out = Z*a - t on DVE; DMA fired right after each piece on
                # sync (p0) / scalar (p1) rings
                for hi in range(2):
                    h0 = 1152 * hi
                    hsl = slice(h0, h0 + 1152)
                    for p in range(2):
                        outp = gout.tile(
                            [128, 1152], BF16, tag=f"outp{p}{hi}", name="outp"
                        )
                        nc.vector.tensor_scalar(
                            outp,
                            Z[p][:, hsl],
                            ats[p][:, 0:1],
                            ats[p][:, 1:2],
                            op0=ALU.mult,
                            op1=ALU.subtract,
                        )
                        ring = nc.sync if p == 0 else nc.scalar
                        ring.dma_start(out=out_d[ts(p, 128), hsl], in_=outp)

    nc.finalize()
    return nc


def _get_nc():
    if "nc" not in _BUILD_CACHE:
        _BUILD_CACHE["nc"] = _build_nc()
    return _BUILD_CACHE["nc"]


def _make_in_maps(inputs):
    import ml_dtypes

    f1 = np.ascontiguousarray(
        np.asarray(inputs["features1"], dtype=np.float32)
        .reshape(B, C, N)
        .astype(ml_dtypes.bfloat16)
    )
    f2 = np.ascontiguousarray(
        np.asarray(inputs["features2"], dtype=np.float32)
        .reshape(B, C, N)
        .astype(ml_dtypes.bfloat16)
    )

    def g(k):
        return np.asarray(inputs[k], dtype=np.float32)

    inv_cnt = 1.0 / (32.0 * N)
    gsel = np.zeros((128, 2 * GROUPS), np.float32)
    # gselTw: rows 0-7 one-hot * gn_w, row 8 = gn_b (col layout [t*128+c])
    gselTw = np.zeros((GROUPS + 1, 2 * 128), np.float32)
    gnw, gnb = g("gn_w"), g("gn_b")
    for t in range(2):
        for gl in range(4):
            grp = 4 * t + gl
            gsel[gl * 32 : (gl + 1) * 32, GROUPS * t + grp] = inv_cnt
            cs = 128 * t + gl * 32
            gselTw[grp, cs : cs + 32] = gnw[grp * 32 : (grp + 1) * 32]
        gselTw[GROUPS, 128 * t : 128 * (t + 1)] = gnb[128 * t : 128 * (t + 1)]
    ms2_init = np.zeros((GROUPS + 1, 2), np.float32)
    ms2_init[GROUPS] = [0.0, -1.0]

    qw, kw, vw = g("q_w"), g("k_w"), g("v_w")
    HT = np.ascontiguousarray((qw.T @ kw).astype(ml_dtypes.bfloat16))
    rcol = (kw.T @ g("q_b")) * SCALE  # [C]
    vwT_aug = np.concatenate([vw.T, rcol[:, None]], axis=1)
    vb_bcast = np.zeros((128, VW), np.float32)
    vb_bcast[:, :O] = g("v_b")[None, :]
    shared = {
        "HT": HT,
        "vwT_aug": np.ascontiguousarray(vwT_aug.astype(ml_dtypes.bfloat16)),
        "owT": np.ascontiguousarray(g("o_w").T.astype(ml_dtypes.bfloat16)),
        "vb_bcast": vb_bcast,
        "o_b": g("o_b"),
        "gsel": gsel,
        "gselTw": np.ascontiguousarray(gselTw.astype(ml_dtypes.bfloat16)),
        "ms2_init": np.ascontiguousarray(ms2_init.astype(ml_dtypes.bfloat16)),
        "ones_bf": np.ones((128, 1), ml_dtypes.bfloat16),
        "magic": np.full((GROUPS, 1), 0x5F3759DF, np.int32),
    }
    return [{"features1": f1[i], "features2": f2[i], **shared} for i in range(B)]


def run(inputs, trace=False):
    from concourse.bass_utils import run_bass_kernel_spmd

    nc = _get_nc()
    in_maps = _make_in_maps(inputs)
    res = run_bass_kernel_spmd(nc, in_maps, core_ids=list(range(B)), trace=trace)
    out = np.stack(
        [np.asarray(res.results[i]["out"]).astype(np.float32) for i in range(B)]
    )
    return out.reshape(B, O, 48, 48), res


def kernel(**inputs):
    out, _ = run(inputs, trace=False)
    return out
